# revision 1
# baseline (speedup 1.0000x reference)
"""BCOP (block-convolution orthogonal parameterization) forward on 8 TRN2 cores.

Math (validated vs reference in fp32 numpy):
  - power iteration via repeated squaring: G = A^T A; with v1 = A^T u0,
    d0 = v1.(G^18 v1), d1 = v1.(G^19 v1) reproduce the reference's
    normalized-power-iteration sigma: s = sqrt(d1/d0); G^18 v1 = G16@(G2@v1).
  - W0 = A/s; 20 Bjorck iters maintaining W and WT = W^T:
      G = W^T W  (lhsT=W,rhs=W);  M = 1.5 I - 0.5 G  (symmetric)
      W' = W M   (lhsT=WT,rhs=M); WT' = M WT (lhsT=M,rhs=WT)
  - downstream needs only WT = ortho^T:
      Z^T_i = WT[i+1] with rows>=128 zeroed -> PQ_i = matmul(lhsT=Z^T,rhs=Z^T)
      b1T[i1,j1] = (block_orth(PQ0,PQ1)[i1,j1])^T and b2 = block_orth(PQ2,PQ3),
      all products of symmetric matrices -> no transposes needed.
      p3[i,j] = sum b1[i1,j1] @ b2[i-i1,j-j1] = matmul(lhsT=b1T[..], rhs=b2[..])
      p_e[i,j] = H @ p3[i,j] = matmul(lhsT=WT[0], rhs=p3[i,j])
  - conv tap (kh,kw) uses stationary lhsT[ci,co] = p_e[kw,kh]; x circularly
    padded to 66x66 in SBUF; 18 accumulating matmuls (9 taps x 2 ci-tiles)
    per [128co x 512px] PSUM tile; bias added on the PSUM->SBUF evacuation.

Parallelization: the per-matrix weight construction (power iteration + Bjorck)
is INDEPENDENT across the 5 parameter matrices, so it is sharded across cores
via the inputs (core i receives matrix i mod 5 only), then an 8-rank AllGather
shares the 5 WT results; assembly + conv then run replicated. The conv is
data-parallel over batch (4 images per core).

PSUM discipline: every accumulation group owns a whole bank; a [128,512] tile
holds both 128-row output halves of a 256x256 product as ONE group.
"""

import numpy as np

import concourse.bass as bass
import concourse.mybir as mybir
import concourse.tile as tile
from concourse import bacc
from concourse.bass_utils import run_bass_kernel_spmd

P = 128
C = 256
NK = 5
N_CORES = 8
B_TOTAL = 32
B_CORE = B_TOTAL // N_CORES
H = 64
PH = 66
NPIX = H * H
BJORCK_ITERS = 20

F32 = mybir.dt.float32
F32R = mybir.dt.float32r
ALU = mybir.AluOpType
ACTF = mybir.ActivationFunctionType


def build_body(tc, out_ap, xs, pmk, u0k, bias_ap, ctx):
    nc = tc.nc
    from concourse.masks import make_identity

    persist = ctx.enter_context(tc.tile_pool(name="persist", bufs=1))
    small = ctx.enter_context(tc.tile_pool(name="small", bufs=3))

    U_sb = persist.tile([P, 2, 1], F32R)
    for tr in range(2):
        nc.sync.dma_start(U_sb[:, tr, :], u0k[tr * P:(tr + 1) * P, :])
    bias_sb = persist.tile([P, 2, 1], F32)
    for mt in range(2):
        nc.sync.dma_start(bias_sb[:, mt, :], bias_ap[mt * P:(mt + 1) * P].unsqueeze(1))

    # ---- constants ----
    ID1 = persist.tile([P, P], F32)
    make_identity(nc, ID1)
    I15 = persist.tile([P, 2, C], F32)
    I10 = persist.tile([P, 2, C], F32)
    nc.vector.memset(I15[:], 0.0)
    nc.vector.memset(I10[:], 0.0)
    for mt in range(2):
        nc.scalar.mul(I15[:, mt, mt * P:(mt + 1) * P], ID1[:], 1.5)
        nc.scalar.mul(I10[:, mt, mt * P:(mt + 1) * P], ID1[:], 1.0)
    I15f = I15.rearrange("p a b -> p (a b)")

    RB = persist.tile([P, 1], F32)            # broadcast 1/s (own matrix)
    Wc_sb = persist.tile([P, 9, 2, C], F32R)  # final conv lhsT tiles, slot (i,j)
    WTfin = persist.tile([P, NK, 2, C], F32R)  # gathered ortho^T, all matrices

    def flat(ap3):
        return ap3.rearrange("p a b -> p (a b)")

    def prod_mms(out_ps, X3, Y3, n_tr=2):
        """dst[mt] += X[tr][:, mt]^T @ Y[tr]; ONE accumulation group per bank."""
        first = True
        for mt in range(2):
            for tr in range(n_tr):
                last = (mt == 1 and tr == n_tr - 1)
                nc.tensor.matmul(out_ps[:, mt * C:(mt + 1) * C],
                                 X3[:, tr, mt * P:(mt + 1) * P], Y3[:, tr, :],
                                 start=first, stop=last)
                first = False

    xpool = ctx.enter_context(tc.tile_pool(name="xpool", bufs=4))

    with tc.tile_pool(name="build", bufs=1) as build, \
         tc.tile_pool(name="wstate", bufs=2) as wpool, \
         tc.tile_pool(name="mpool", bufs=4) as mpool, \
         tc.tile_pool(name="vpool", bufs=8) as vpool, \
         tc.tile_pool(name="ccdram", bufs=1, space="DRAM") as ccdram:

        Wcur = wpool.tile([P, 2, C], F32R, tag="W")
        WTcur = wpool.tile([P, 2, C], F32R, tag="WT")
        G_sb = build.tile([P, 2, C], F32R)
        G2_sb = build.tile([P, 2, C], F32R)
        G16_sb = build.tile([P, 2, C], F32R)

        for tr in range(2):
            nc.sync.dma_start(Wcur[:, tr, :], pmk[tr * P:(tr + 1) * P, :])

        # ============ phase 1: sigma via repeated squaring (own matrix) ======
        with tc.tile_pool(name="ps1", bufs=2, space="PSUM") as ps1:
            gps = ps1.tile([P, 2 * C], F32, tag="sq")
            prod_mms(gps, Wcur, Wcur)
            nc.scalar.copy(flat(G_sb), gps[:])

            prev = G_sb
            for pw in (2, 4, 8, 16):
                sq = ps1.tile([P, 2 * C], F32, tag="sq", name=f"sq{pw}")
                prod_mms(sq, prev, prev)
                if pw == 2:
                    dst = G2_sb
                elif pw == 16:
                    dst = G16_sb
                else:
                    dst = build.tile([P, 2, C], F32R, tag="gtmp",
                                     name=f"g{pw}", bufs=2)
                if pw in (4, 16):
                    nc.vector.tensor_copy(flat(dst), sq[:])
                else:
                    nc.scalar.copy(flat(dst), sq[:])
                prev = dst

            def matvec(G3, vin, nm):
                vout = vpool.tile([P, 2], F32R, tag="v", name=f"v_{nm}")
                for mt in range(2):
                    vps = ps1.tile([P, 1], F32, tag="vps", bufs=4,
                                   name=f"vp_{nm}_{mt}")
                    for tr in range(2):
                        nc.tensor.matmul(
                            vps[:], G3[:, tr, mt * P:(mt + 1) * P].bitcast(F32),
                            vin[:, tr:tr + 1].bitcast(F32),
                            start=(tr == 0), stop=(tr == 1))
                    nc.scalar.copy(vout[:, mt:mt + 1], vps[:])
                return vout

            v1 = matvec(Wcur, U_sb, "v1")
            m1 = matvec(G2_sb, v1, "m1")
            m2 = matvec(G16_sb, m1, "m2")
            m3 = matvec(G_sb, m2, "m3")

            def dot(va, vb, nm):
                dps = ps1.tile([1, 1], F32, tag="vps", bufs=4, name=f"d_{nm}")
                for tr in range(2):
                    nc.tensor.matmul(dps[:], va[:, tr:tr + 1].bitcast(F32),
                                     vb[:, tr:tr + 1].bitcast(F32),
                                     start=(tr == 0), stop=(tr == 1))
                return dps

            dps0 = dot(v1, m2, "0")
            dps1 = dot(v1, m3, "1")
            dsb = small.tile([1, 3], F32, tag="dsb")
            nc.vector.tensor_copy(dsb[:, 0:1], dps0[:])
            nc.vector.reciprocal(dsb[:, 1:2], dps1[:])
            nc.vector.tensor_mul(dsb[:, 2:3], dsb[:, 0:1], dsb[:, 1:2])
            rsb = small.tile([1, 1], F32, tag="rsb")
            nc.scalar.sqrt(rsb[:], dsb[:, 2:3])
            nc.gpsimd.partition_broadcast(RB[:, 0:1], rsb[:])
            # W0 = A * r (in place), then WT0 = W0^T via PE transpose
            nc.vector.tensor_scalar_mul(Wcur[:], Wcur[:], RB[:, 0:1])
            for tr in range(2):
                for mt in range(2):
                    tps = ps1.tile([P, P], F32, tag="tp")
                    nc.tensor.transpose(
                        tps[:], Wcur[:, tr, mt * P:(mt + 1) * P].bitcast(F32),
                        ID1[:])
                    nc.scalar.copy(WTcur[:, mt, tr * P:(tr + 1) * P], tps[:])

        # ================= phase 2: Bjorck (own matrix) =================
        with tc.tile_pool(name="ps2", bufs=2, space="PSUM") as ps2:
            for it in range(BJORCK_ITERS):
                last = it == BJORCK_ITERS - 1
                Wnxt = None if last else wpool.tile([P, 2, C], F32R, tag="W",
                                                    name=f"W_{it}")
                WTnxt = wpool.tile([P, 2, C], F32R, tag="WT", name=f"WT_{it}")
                gps = ps2.tile([P, 2 * C], F32, tag="g", bufs=2)
                prod_mms(gps, Wcur, Wcur)
                m_sb = mpool.tile([P, 2 * C], F32R, tag="m", name=f"m_{it}")
                nc.vector.scalar_tensor_tensor(
                    m_sb[:], gps[:], -0.5, I15f, op0=ALU.mult, op1=ALU.add)
                m3 = m_sb.rearrange("p (a b) -> p a b", b=C)
                if not last:
                    wps = ps2.tile([P, 2 * C], F32, tag="w", bufs=2)
                    prod_mms(wps, WTcur, m3)
                    nc.scalar.copy(flat(Wnxt), wps[:])
                wtps = ps2.tile([P, 2 * C], F32, tag="wt", bufs=2)
                prod_mms(wtps, m3, WTcur)
                nc.vector.tensor_copy(flat(WTnxt), wtps[:])
                if Wnxt is not None:
                    Wcur = Wnxt
                WTcur = WTnxt

        # ============ AllGather the needed WT halves across cores ============
        # Assembly reads only row-tile 0 of WT[1..4] (the masked projections)
        # but both row-tiles of WT[0] (H). Cores 0-4 contribute their tr=0
        # half; core 5 (a k=0 duplicate) contributes k=0's tr=1 half via a
        # partition-id-predicated DMA. Halves the AllGather payload.
        cc_in = ccdram.tile([1, P * C], F32R)
        cc_out = ccdram.tile([N_CORES, P * C], F32R, addr_space="Shared")
        pid = nc.sync.partition_id()
        nc.sync.dma_start(cc_in[0].rearrange("(p n) -> p n", p=P),
                          WTcur[:, 0, :], cond=(pid != 5))
        nc.sync.dma_start(cc_in[0].rearrange("(p n) -> p n", p=P),
                          WTcur[:, 1, :], cond=(pid == 5))
        nc.gpsimd.collective_compute(
            "AllGather", ALU.bypass, ins=[cc_in.opt()], outs=[cc_out.opt()],
            replica_groups=[list(range(N_CORES))])
        for k in range(NK):
            nc.sync.dma_start(WTfin[:, k, 0, :],
                              cc_out[k].rearrange("(p n) -> p n", p=P))
        nc.sync.dma_start(WTfin[:, 0, 1, :],
                          cc_out[NK].rearrange("(p n) -> p n", p=P))



        # ================= phase 3: weight assembly =================
        PQ_sb = build.tile([P, 4, 2, C], F32R)
        IP_sb = build.tile([P, 4, 2, C], F32R)
        b1T_sb = build.tile([P, 2, 2, 2, C], F32R)
        b2_sb = build.tile([P, 2, 2, 2, C], F32R)
        p3_sb = build.tile([P, 9, 2, C], F32R)

        with tc.tile_pool(name="ps3", bufs=4, space="PSUM") as ps3:
            for i in range(4):
                qps = ps3.tile([P, 2 * C], F32, tag="as", name=f"q_{i}")
                prod_mms(qps, WTfin[:, i + 1], WTfin[:, i + 1], n_tr=1)
                nc.scalar.copy(flat(PQ_sb[:, i]), qps[:])
                nc.vector.tensor_sub(flat(IP_sb[:, i]), flat(I10), flat(PQ_sb[:, i]))

            def symprod(dst3, X3, Y3, nm):
                bps = ps3.tile([P, 2 * C], F32, tag="as", name=f"bp_{nm}")
                prod_mms(bps, X3, Y3)
                nc.scalar.copy(flat(dst3), bps[:])

            for i1 in range(2):
                for j1 in range(2):
                    symprod(b1T_sb[:, i1, j1],
                            IP_sb[:, 1] if j1 else PQ_sb[:, 1],
                            IP_sb[:, 0] if i1 else PQ_sb[:, 0], f"b1T{i1}{j1}")
            for i2 in range(2):
                for j2 in range(2):
                    symprod(b2_sb[:, i2, j2],
                            IP_sb[:, 2] if i2 else PQ_sb[:, 2],
                            IP_sb[:, 3] if j2 else PQ_sb[:, 3], f"b2{i2}{j2}")

            for i in range(3):
                for j in range(3):
                    terms = [(i1, j1) for i1 in range(2) for j1 in range(2)
                             if 0 <= i - i1 < 2 and 0 <= j - j1 < 2]
                    pps = ps3.tile([P, 2 * C], F32, tag="as", name=f"p3_{i}{j}")
                    nmm = len(terms) * 4
                    idx = 0
                    for mt in range(2):
                        for (i1, j1) in terms:
                            for tr in range(2):
                                idx += 1
                                nc.tensor.matmul(
                                    pps[:, mt * C:(mt + 1) * C],
                                    b1T_sb[:, i1, j1, tr, mt * P:(mt + 1) * P],
                                    b2_sb[:, i - i1, j - j1, tr, :],
                                    start=(idx == 1), stop=(idx == nmm))
                    nc.vector.tensor_copy(flat(p3_sb[:, 3 * i + j]), pps[:])

            for i in range(3):
                for j in range(3):
                    eps_ = ps3.tile([P, 2 * C], F32, tag="as", name=f"pe_{i}{j}")
                    prod_mms(eps_, WTfin[:, 0], p3_sb[:, 3 * i + j])
                    nc.scalar.copy(flat(Wc_sb[:, 3 * i + j]), eps_[:])

    # ================= phase 4: conv =================
    with tc.tile_pool(name="opool", bufs=3) as opool, \
         tc.tile_pool(name="psC", bufs=6, space="PSUM") as psC:
        for b in range(B_CORE):
            xp = []
            for tr in range(2):
                xpt = xpool.tile([P, PH, PH], F32R, tag="xp", name=f"xp_{b}_{tr}")
                nc.sync.dma_start(xpt[:, 1:65, 1:65], xs[b, tr * P:(tr + 1) * P, :, :])
                nc.vector.tensor_copy(xpt[:, 1:65, 0:1], xpt[:, 1:65, 64:65])
                nc.vector.tensor_copy(xpt[:, 1:65, 65:66], xpt[:, 1:65, 1:2])
                nc.vector.tensor_copy(xpt[:, 0:1, :], xpt[:, 64:65, :])
                nc.vector.tensor_copy(xpt[:, 65:66, :], xpt[:, 1:2, :])
                xp.append(xpt)
            for mt in range(2):
                osb = opool.tile([P, NPIX], F32, tag="osb", name=f"osb_{b}_{mt}")
                for pb in range(8):
                    ops = psC.tile([P, 512], F32, tag="o")
                    idx = 0
                    for kh in range(3):
                        for kw in range(3):
                            for tr in range(2):
                                nc.tensor.matmul(
                                    ops[:],
                                    Wc_sb[:, 3 * kw + kh, tr, mt * P:(mt + 1) * P],
                                    xp[tr][:, pb * 8 + kh:pb * 8 + kh + 8,
                                           kw:kw + 64],
                                    start=(idx == 0), stop=(idx == 17))
                                idx += 1
                    nc.scalar.activation(
                        osb[:, pb * 512:(pb + 1) * 512], ops[:], ACTF.Identity,
                        bias=bias_sb[:, mt, :], scale=1.0)
                    nc.sync.dma_start(
                        out_ap[b, mt * P:(mt + 1) * P, pb * 8:(pb + 1) * 8, :],
                        osb[:, pb * 512:(pb + 1) * 512].rearrange(
                            "p (h w) -> p h w", w=H))


def build_program():
    from contextlib import ExitStack
    nc = bacc.Bacc("TRN2", target_bir_lowering=False, debug=False,
                   enable_asserts=False, num_devices=N_CORES)
    xs = nc.dram_tensor("xs", [B_CORE, C, H, H], F32R, kind="ExternalInput").ap()
    pmk = nc.dram_tensor("pmk", [C, C], F32R, kind="ExternalInput").ap()
    u0k = nc.dram_tensor("u0k", [C, 1], F32R, kind="ExternalInput").ap()
    bias = nc.dram_tensor("bias", [C], F32, kind="ExternalInput").ap()
    out = nc.dram_tensor("out", [B_CORE, C, H, H], F32, kind="ExternalOutput").ap()
    with tile.TileContext(nc) as tc:
        with ExitStack() as ctx:
            build_body(tc, out, xs, pmk, u0k, bias, ctx)
    nc.compile()
    return nc


_cached_nc = None


def make_in_maps(x, pm, u0, b):
    in_maps = []
    for i in range(N_CORES):
        k = i if i < NK else i - NK
        in_maps.append({
            "xs": np.ascontiguousarray(x[i * B_CORE:(i + 1) * B_CORE]),
            "pmk": np.ascontiguousarray(pm[k]),
            "u0k": np.ascontiguousarray(u0[k]),
            "bias": np.ascontiguousarray(b),
        })
    return in_maps


def kernel(x, param_matrices, init_u, bias):
    global _cached_nc
    if _cached_nc is None:
        _cached_nc = build_program()
    nc = _cached_nc
    x = np.ascontiguousarray(np.asarray(x, dtype=np.float32))
    pm = np.ascontiguousarray(np.asarray(param_matrices, dtype=np.float32))
    u0 = np.ascontiguousarray(np.asarray(init_u, dtype=np.float32))
    b = np.ascontiguousarray(np.asarray(bias, dtype=np.float32))
    in_maps = make_in_maps(x, pm, u0, b)
    res = run_bass_kernel_spmd(nc, in_maps, core_ids=list(range(N_CORES)))
    return np.concatenate([r["out"] for r in res.results], axis=0)


if __name__ == "__main__":
    import reference
    inputs = {k: np.asarray(v) for k, v in reference.setup_inputs().items()}
    out = kernel(**inputs)
    print(out.shape, out.dtype)

